# revision 37
# baseline (speedup 1.0000x reference)
"""Colorizer kernel for Trainium2 (8 NeuronCores, SPMD).

out[b,c,y,x] = sum_p softmax_p(corr[b,p,y,x]) * one_hot(labels)[c, y+dy, x+dx]
over a 13x13 displacement window; corr = <feats_t[:,y,x], feats_r[:,y+dy,x+dx]>
over 256 channels; out-of-bounds displacements get zero weight.

Sharding: core = half*4 + batch. Each core: 64 query rows. The bottom half is
y-MIRRORED on host so all 8 cores run one identical SPMD program (the 13x13
window and band mask are y-symmetric).

Pipeline per core (72 key rows = 9 block-rows x 8 x-blocks of 8x16 keys):
  t:    fp16, 2 contiguous SBUF tiles [128ch, 64*128]; gram matmuls read
        strided query windows directly from them (no restage).
  Gram: keys-stationary fp16 matmuls, 2 channel chunks accumulated in PSUM.
  exp:  ScalarE activation (bias -64) PSUM -> bf16 E in SBUF (bf16 keeps the
        fp32 exponent range; exp(corr-64) underflows fp16).
  mask: one VectorE bf16 multiply with the translation-invariant band mask.
  agg:  bf16 matmuls, stationary [128 keys, 16 classes + ones col], PSUM
        accumulation per 8-query-row strip; GpSimd copy to SBUF; DMA num+den
        to HBM; host does the final divide.
"""
import sys
sys.path.insert(0, "/opt/trn_rl_repo")

import numpy as np
import ml_dtypes

D, R, C = 4, 6, 16
B, CF, H1, W1 = 4, 256, 128, 128
HALF = 64
NBR = 9             # key block-rows per core (72 key rows)
NXB = 8             # x-blocks per row (16 key cols each)
BIAS = -64.0
EPAD = 576          # E cols reserved per x-block (max rows*nx = 560)
OW = 32             # oht stationary stride (classes 0..15, ones at 16)

_COMPILED = None
_LAST_RES = None


def _windows():
    out = []
    for k in range(NBR):
        ky0 = 8 * k
        a0 = max(0, ky0 - 6)
        b0 = min(HALF, ky0 + 14)
        rows = b0 - a0
        row = []
        for xb in range(NXB):
            kx0 = 16 * xb
            xlo = max(0, kx0 - 6)
            xhi = min(W1, kx0 + 22)
            nx = xhi - xlo
            if rows * nx <= 512:
                pieces = [(0, rows)]
            else:
                h = rows // 2
                pieces = [(0, h), (h, rows - h)]
            assert all(pr * nx <= 512 for _, pr in pieces)
            row.append(dict(ky0=ky0, a0=a0, b0=b0, rows=rows,
                            xlo=xlo, nx=nx, pieces=pieces))
        out.append(row)
    return out


def _build():
    import concourse.tile as tile
    import concourse.mybir as mybir
    from concourse import bacc
    from contextlib import ExitStack

    f32 = mybir.dt.float32
    fp16 = mybir.dt.float16
    bf16 = mybir.dt.bfloat16
    Exp = mybir.ActivationFunctionType.Exp

    win = _windows()

    nc = bacc.Bacc("TRN2", target_bir_lowering=False, debug=False, num_devices=8)
    t_d = nc.dram_tensor("t", [CF, HALF * W1], fp16, kind="ExternalInput").ap()
    r_d = nc.dram_tensor("r", [CF, 72 * W1], fp16, kind="ExternalInput").ap()
    oht_d = nc.dram_tensor("oht", [128, NBR * NXB * OW], bf16,
                           kind="ExternalInput").ap()
    msk_d = nc.dram_tensor("msk", [128, 32 * 28], bf16, kind="ExternalInput").ap()
    out_d = nc.dram_tensor("out", [C + 1, HALF, W1], f32,
                           kind="ExternalOutput").ap()

    with tile.TileContext(nc) as tc, ExitStack() as ctx:
        const_p = ctx.enter_context(tc.tile_pool(name="const", bufs=1))
        t_p = ctx.enter_context(tc.tile_pool(name="tbuf", bufs=1))
        r_p = ctx.enter_context(tc.tile_pool(name="rbuf", bufs=1))
        e_p = ctx.enter_context(tc.tile_pool(name="ebuf", bufs=4))
        st_p = ctx.enter_context(tc.tile_pool(name="stage", bufs=2))
        gps = ctx.enter_context(tc.tile_pool(name="gram", bufs=3, space="PSUM"))
        aps = ctx.enter_context(tc.tile_pool(name="aggp", bufs=1, space="PSUM"))

        bias_t = const_p.tile([128, 1], f32)
        nc.vector.memset(bias_t[:], BIAS)
        warm_t = const_p.tile([128, 512], fp16)
        nc.vector.memset(warm_t[:], 0.0)
        oht_t = const_p.tile([128, NBR * NXB * OW], bf16)
        msk_t = const_p.tile([128, 32 * 28], bf16)
        msk3 = msk_t[:].rearrange("p (m x) -> p m x", m=32)

        # t: two big contiguous fp16 tiles, one per channel chunk; gram reads
        # strided [rows, nx] windows straight out of them.
        t_t = []
        t3 = []
        for ch in (0, 1):
            tl = t_p.tile([128, HALF * W1], fp16, name=f"t{ch}")
            t_t.append(tl)
            t3.append(tl[:].rearrange("p (h w) -> p h w", h=HALF))
        # r: all 9 block-rows resident (2KB/partition per chunk) -> the input
        # DMA queue never stalls on buffer reuse.
        r_t = [[r_p.tile([128, 8 * W1], fp16, tag=f"r{ch}_{k}",
                         name=f"r{ch}_{k}") for ch in (0, 1)]
               for k in range(NBR)]

        # Input DMAs split across the two HWDGE queues so compute starts
        # ~4us in: sync gets t ch0 + all r; scalar gets consts + t ch1.
        def dma_r(k):
            for ch in (0, 1):
                nc.sync.dma_start(
                    r_t[k][ch][:],
                    r_d[ch * 128:(ch + 1) * 128, k * 8 * W1:(k + 1) * 8 * W1])

        # r0 + t ch1 ride the scalar queue, t ch0 + the rest of r the sync
        # queue; t goes in 16-row pieces interleaved with r so the first
        # gram blocks are gated on ~0.5MB, not a whole channel half.
        QT = HALF * W1 // 4

        def dma_t(eng, ch, q):
            eng.dma_start(t_t[ch][:, q * QT:(q + 1) * QT],
                          t_d[ch * 128:(ch + 1) * 128, q * QT:(q + 1) * QT])

        for ch in (0, 1):
            nc.scalar.dma_start(
                r_t[0][ch][:], r_d[ch * 128:(ch + 1) * 128, 0:8 * W1])
        dma_t(nc.sync, 0, 0)
        dma_t(nc.scalar, 1, 0)
        dma_r(1)
        dma_t(nc.sync, 0, 1)
        dma_t(nc.scalar, 1, 1)
        nc.scalar.dma_start(oht_t[:], oht_d[:])
        dma_r(2)
        dma_t(nc.sync, 0, 2)
        dma_t(nc.scalar, 1, 2)
        nc.scalar.dma_start(msk_t[:], msk_d[:])
        dma_r(3)
        dma_t(nc.sync, 0, 3)
        dma_t(nc.scalar, 1, 3)
        for k in range(4, NBR):
            dma_r(k)

        # Dummy matmuls during the input-DMA window: keeps the PE HAM clock
        # gate open and the p-state ramped so the real stream starts warm.
        wp = gps.tile([128, 1024], f32, tag="G")
        for _ in range(24):
            nc.tensor.matmul(wp[0:16, 0:512], warm_t[:, 0:16], warm_t[:],
                             start=True, stop=True)

        e_tiles = {}

        def do_strip(s):
            pt = aps.tile([32, 1024], f32, tag="aggps")
            pt3 = pt[:].rearrange("p (r x) -> p r x", r=8)
            started = [False, False]
            for k in range(NBR):
                w0 = win[k][0]
                if not (w0['a0'] < 8 * s + 8 and w0['b0'] > 8 * s):
                    continue
                et = e_tiles[k]
                for xb in range(NXB):
                    w = win[k][xb]
                    ra = max(w['a0'], 8 * s)
                    rb = min(w['b0'], 8 * s + 8)
                    if ra >= rb:
                        continue
                    for (pa, pb) in ((ra, min(rb, 8 * s + 4)),
                                     (max(ra, 8 * s + 4), rb)):
                        if pa >= pb:
                            continue
                        bank = (pa - 8 * s) // 4
                        nx = w['nx']
                        rhs = et[:, xb * EPAD + (pa - w['a0']) * nx:
                                 xb * EPAD + (pb - w['a0']) * nx]
                        lin = k * NXB + xb
                        o = pt3[0:C + 1, pa - 8 * s:pb - 8 * s,
                                w['xlo']:w['xlo'] + nx]
                        nc.tensor.matmul(
                            o, oht_t[:, lin * OW:lin * OW + C + 1], rhs,
                            start=not started[bank], stop=False)
                        started[bank] = True
            stg = st_p.tile([C + 1, 1024], f32, tag="stg")
            nc.vector.tensor_copy(stg[:], pt[0:C + 1, :])
            nc.sync.dma_start(
                out_d[:, 8 * s:8 * s + 8, :],
                stg[:].rearrange("p (r x) -> p r x", r=8))

        k_order = list(range(NBR))
        pos = {k: i for i, k in enumerate(k_order)}
        strip_after = {}
        for s in range(HALF // 8):
            ks = [k for k in range(NBR)
                  if win[k][0]['a0'] < 8 * s + 8 and win[k][0]['b0'] > 8 * s]
            strip_after[s] = max(ks, key=lambda k: pos[k])

        for k in k_order:
            # k=8 lives until strip 7 at the very end -> own buffer, not the
            # rotating pool (a rotating slot would stall later E allocations).
            et = e_p.tile([128, NXB * EPAD], bf16,
                          tag="E8" if k == 8 else "E")
            e_tiles[k] = et
            for xb in range(NXB):
                w = win[k][xb]
                rows, nx, xlo, a0 = w['rows'], w['nx'], w['xlo'], w['a0']
                ntot = rows * nx
                gp = gps.tile([128, 1024], f32, tag="G")
                for ch in (0, 1):
                    lhsT = r_t[k][ch][:, 128 * xb:128 * xb + 128]
                    for pi, (off, pr) in enumerate(w['pieces']):
                        rhs = t3[ch][:, a0 + off:a0 + off + pr, xlo:xlo + nx]
                        o = gp[:, pi * 512:pi * 512 + pr * nx]
                        nc.tensor.matmul(o, lhsT, rhs, start=(ch == 0),
                                         stop=(ch == 1))
                eo = et[:, xb * EPAD:xb * EPAD + ntot]
                if len(w['pieces']) == 1:
                    ei = gp[:, 0:ntot]
                else:
                    h = w['pieces'][0][1] * nx
                    ei = gp[:].rearrange("p (t c) -> p t c", t=2)[:, :, 0:h]
                    eo = eo.rearrange("p (t c) -> p t c", t=2)
                nc.scalar.activation(eo, ei, Exp, bias=bias_t[:], scale=1.0)
                m_a = a0 - w['ky0'] + 12
                xr = xlo - (16 * xb - 6)
                e3 = et[:, xb * EPAD:xb * EPAD + ntot].rearrange(
                    "p (r x) -> p r x", r=rows)
                nc.vector.tensor_mul(
                    e3, e3, msk3[:, m_a:m_a + rows, xr:xr + nx])
            for s in range(HALF // 8):
                if strip_after[s] == k:
                    do_strip(s)
    nc.compile()
    return nc


def _prep_host(quantized_r):
    q = quantized_r[:, 0]
    a = q.reshape(B, H1, 4, 512)[:, :, 1:3, :].sum(2)
    s = a.reshape(B, H1, W1, 4)[:, :, :, 1:3].sum(3)
    # CPU-jax reference semantics: f32->i32 convert truncates (values >= 0)
    return s // 4


def _mask_host():
    ky = (np.arange(128) // 16)[:, None, None]
    kx = (np.arange(128) % 16)[:, None, None]
    mi = np.arange(32)[None, :, None]
    rx = np.arange(28)[None, None, :]
    m = ((np.abs(mi - 12 - ky) <= 6) & (np.abs(rx - 6 - kx) <= 6))
    return m.astype(np.float32).reshape(128, 32 * 28).astype(ml_dtypes.bfloat16)


def _oht_host(labels_half):
    o = np.zeros((128, NBR * NXB, OW), np.float32)
    for k in range(NBR):
        for xb in range(NXB):
            lab = labels_half[8 * k:8 * k + 8, 16 * xb:16 * xb + 16].reshape(128)
            o[np.arange(128), k * NXB + xb, lab] = 1.0
            o[:, k * NXB + xb, C] = 1.0  # denominator ones column
    return o.reshape(128, NBR * NXB * OW).astype(ml_dtypes.bfloat16)


def kernel(feats_r, feats_t, quantized_r):
    global _COMPILED, _LAST_RES
    import os
    from concourse.bass_utils import run_bass_kernel_spmd

    feats_r = np.asarray(feats_r, np.float32)
    feats_t = np.asarray(feats_t, np.float32)
    quantized_r = np.asarray(quantized_r, np.int32)

    if _COMPILED is None:
        _COMPILED = _build()

    labels = _prep_host(quantized_r)
    msk = _mask_host()
    fr16 = feats_r.astype(np.float16)
    ft16 = feats_t.astype(np.float16)
    in_maps = []
    for core in range(8):
        half, b = core // 4, core % 4
        if half == 0:
            t = ft16[b, :, 0:HALF, :]
            r = fr16[b, :, 0:72, :]
            lab = labels[b, 0:72, :]
        else:  # y-mirrored bottom half
            t = ft16[b, :, ::-1, :][:, 0:HALF, :]
            r = fr16[b, :, ::-1, :][:, 0:72, :]
            lab = labels[b, ::-1, :][0:72, :]
        r_bm = np.ascontiguousarray(r).reshape(CF, NBR, 8, NXB, 16) \
            .transpose(0, 1, 3, 2, 4).reshape(CF, 72 * W1)
        in_maps.append(dict(
            t=np.ascontiguousarray(t.reshape(CF, HALF * W1)),
            r=np.ascontiguousarray(r_bm),
            oht=np.ascontiguousarray(_oht_host(lab)),
            msk=msk,
        ))
    trace = bool(os.environ.get("KPROF"))
    res = run_bass_kernel_spmd(_COMPILED, in_maps, core_ids=list(range(8)),
                               trace=trace)
    _LAST_RES = res
    out = np.empty((B, C, H1, W1), np.float32)
    for core in range(8):
        half, b = core // 4, core % 4
        o = res.results[core]["out"]
        v = o[0:C] / o[C:C + 1]
        if half == 0:
            out[b, :, 0:HALF, :] = v
        else:
            out[b, :, HALF:, :] = v[:, ::-1, :]
    return out


# revision 39
# speedup vs baseline: 1.2127x; 1.2127x over previous
"""Colorizer kernel for Trainium2 (8 NeuronCores, SPMD).

out[b,c,y,x] = sum_p softmax_p(corr[b,p,y,x]) * one_hot(labels)[c, y+dy, x+dx]
over a 13x13 displacement window; corr = <feats_t[:,y,x], feats_r[:,y+dy,x+dx]>
over 256 channels; out-of-bounds displacements get zero weight.

Sharding: core = half*4 + batch. Each core: 64 query rows. The bottom half is
y-MIRRORED on host so all 8 cores run one identical SPMD program (the 13x13
window and band mask are y-symmetric).

Pipeline per core (72 key rows = 9 block-rows x 8 x-blocks of 8x16 keys):
  t:    fp16, 2 contiguous SBUF tiles [128ch, 64*128]; gram matmuls read
        strided query windows directly from them (no restage).
  Gram: keys-stationary fp16 matmuls, 2 channel chunks accumulated in PSUM.
  exp:  ScalarE activation (bias -64) PSUM -> bf16 E in SBUF (bf16 keeps the
        fp32 exponent range; exp(corr-64) underflows fp16).
  mask: one VectorE bf16 multiply with the translation-invariant band mask.
  agg:  bf16 matmuls, stationary [128 keys, 16 classes + ones col], PSUM
        accumulation per 8-query-row strip; GpSimd copy to SBUF; DMA num+den
        to HBM; host does the final divide.
"""
import sys
sys.path.insert(0, "/opt/trn_rl_repo")

import numpy as np
import ml_dtypes

D, R, C = 4, 6, 16
B, CF, H1, W1 = 4, 256, 128, 128
HALF = 64
NBR = 9             # key block-rows per core (72 key rows)
NXB = 8             # x-blocks per row (16 key cols each)
BIAS = -64.0
EPAD = 576          # E cols reserved per x-block (max rows*nx = 560)
OW = 32             # oht stationary stride (classes 0..15, ones at 16)

_COMPILED = None
_LAST_RES = None


def _windows():
    out = []
    for k in range(NBR):
        ky0 = 8 * k
        a0 = max(0, ky0 - 6)
        b0 = min(HALF, ky0 + 14)
        rows = b0 - a0
        row = []
        for xb in range(NXB):
            kx0 = 16 * xb
            xlo = max(0, kx0 - 6)
            xhi = min(W1, kx0 + 22)
            nx = xhi - xlo
            if rows * nx <= 512:
                pieces = [(0, rows)]
            else:
                h = rows // 2
                pieces = [(0, h), (h, rows - h)]
            assert all(pr * nx <= 512 for _, pr in pieces)
            row.append(dict(ky0=ky0, a0=a0, b0=b0, rows=rows,
                            xlo=xlo, nx=nx, pieces=pieces))
        out.append(row)
    return out


def _build():
    import concourse.tile as tile
    import concourse.mybir as mybir
    from concourse import bacc
    from contextlib import ExitStack

    f32 = mybir.dt.float32
    fp16 = mybir.dt.float16
    bf16 = mybir.dt.bfloat16
    Exp = mybir.ActivationFunctionType.Exp

    win = _windows()

    nc = bacc.Bacc("TRN2", target_bir_lowering=False, debug=False, num_devices=8)
    t_d = nc.dram_tensor("t", [CF, HALF * W1], fp16, kind="ExternalInput").ap()
    r_d = nc.dram_tensor("r", [CF, 72 * W1], fp16, kind="ExternalInput").ap()
    oht_d = nc.dram_tensor("oht", [128, NBR * NXB * OW], bf16,
                           kind="ExternalInput").ap()
    msk_d = nc.dram_tensor("msk", [128, 32 * 28], bf16, kind="ExternalInput").ap()
    out_d = nc.dram_tensor("out", [C + 1, HALF, W1], f32,
                           kind="ExternalOutput").ap()

    with tile.TileContext(nc) as tc, ExitStack() as ctx:
        const_p = ctx.enter_context(tc.tile_pool(name="const", bufs=1))
        t_p = ctx.enter_context(tc.tile_pool(name="tbuf", bufs=1))
        r_p = ctx.enter_context(tc.tile_pool(name="rbuf", bufs=1))
        e_p = ctx.enter_context(tc.tile_pool(name="ebuf", bufs=4))
        st_p = ctx.enter_context(tc.tile_pool(name="stage", bufs=2))
        gps = ctx.enter_context(tc.tile_pool(name="gram", bufs=3, space="PSUM"))
        aps = ctx.enter_context(tc.tile_pool(name="aggp", bufs=1, space="PSUM"))

        bias_t = const_p.tile([128, 1], f32)
        nc.vector.memset(bias_t[:], BIAS)
        warm_t = const_p.tile([128, 512], fp16)
        nc.vector.memset(warm_t[:], 0.0)
        oht_t = const_p.tile([128, NBR * NXB * OW], bf16)
        msk_t = const_p.tile([128, 32 * 28], bf16)
        msk3 = msk_t[:].rearrange("p (m x) -> p m x", m=32)

        # t: two big contiguous fp16 tiles, one per channel chunk; gram reads
        # strided [rows, nx] windows straight out of them.
        t_t = []
        t3 = []
        for ch in (0, 1):
            tl = t_p.tile([128, HALF * W1], fp16, name=f"t{ch}")
            t_t.append(tl)
            t3.append(tl[:].rearrange("p (h w) -> p h w", h=HALF))
        # r: all 9 block-rows resident (2KB/partition per chunk) -> the input
        # DMA queue never stalls on buffer reuse.
        r_t = [[r_p.tile([128, 8 * W1], fp16, tag=f"r{ch}_{k}",
                         name=f"r{ch}_{k}") for ch in (0, 1)]
               for k in range(NBR)]

        # Input DMAs split across the two HWDGE queues so compute starts
        # ~4us in: sync gets t ch0 + all r; scalar gets consts + t ch1.
        def dma_r(k):
            for ch in (0, 1):
                nc.sync.dma_start(
                    r_t[k][ch][:],
                    r_d[ch * 128:(ch + 1) * 128, k * 8 * W1:(k + 1) * 8 * W1])

        HW2 = HALF * W1 // 2
        # r0 + t ch1 ride the scalar queue, t ch0 + the rest of r the sync
        # queue, so both HBM streams run concurrently and the first gram
        # block is gated on whichever a-half lands last.
        for ch in (0, 1):
            nc.scalar.dma_start(
                r_t[0][ch][:], r_d[ch * 128:(ch + 1) * 128, 0:8 * W1])
        nc.sync.dma_start(t_t[0][:, 0:HW2], t_d[0:128, 0:HW2])
        nc.scalar.dma_start(t_t[1][:, 0:HW2], t_d[128:256, 0:HW2])
        nc.scalar.dma_start(oht_t[:], oht_d[:])
        nc.scalar.dma_start(msk_t[:], msk_d[:])
        dma_r(1)
        dma_r(2)
        nc.sync.dma_start(t_t[0][:, HW2:], t_d[0:128, HW2:])
        nc.scalar.dma_start(t_t[1][:, HW2:], t_d[128:256, HW2:])
        for k in range(3, NBR):
            dma_r(k)

        # Dummy matmuls during the input-DMA window: keeps the PE HAM clock
        # gate open and the p-state ramped so the real stream starts warm.
        wp = gps.tile([128, 1024], f32, tag="G")
        for _ in range(10):
            nc.tensor.matmul(wp[0:16, 0:512], warm_t[:, 0:16], warm_t[:],
                             start=True, stop=True)

        e_tiles = {}

        def do_strip(s):
            pt = aps.tile([32, 1024], f32, tag="aggps")
            pt3 = pt[:].rearrange("p (r x) -> p r x", r=8)
            started = [False, False]
            for k in range(NBR):
                w0 = win[k][0]
                if not (w0['a0'] < 8 * s + 8 and w0['b0'] > 8 * s):
                    continue
                et = e_tiles[k]
                for xb in range(NXB):
                    w = win[k][xb]
                    ra = max(w['a0'], 8 * s)
                    rb = min(w['b0'], 8 * s + 8)
                    if ra >= rb:
                        continue
                    for (pa, pb) in ((ra, min(rb, 8 * s + 4)),
                                     (max(ra, 8 * s + 4), rb)):
                        if pa >= pb:
                            continue
                        bank = (pa - 8 * s) // 4
                        nx = w['nx']
                        rhs = et[:, xb * EPAD + (pa - w['a0']) * nx:
                                 xb * EPAD + (pb - w['a0']) * nx]
                        lin = k * NXB + xb
                        o = pt3[0:C + 1, pa - 8 * s:pb - 8 * s,
                                w['xlo']:w['xlo'] + nx]
                        nc.tensor.matmul(
                            o, oht_t[:, lin * OW:lin * OW + C + 1], rhs,
                            start=not started[bank], stop=False)
                        started[bank] = True
            stg = st_p.tile([C + 1, 1024], f32, tag="stg")
            nc.vector.tensor_copy(stg[:], pt[0:C + 1, :])
            nc.sync.dma_start(
                out_d[:, 8 * s:8 * s + 8, :],
                stg[:].rearrange("p (r x) -> p r x", r=8))

        k_order = list(range(NBR))
        pos = {k: i for i, k in enumerate(k_order)}
        strip_after = {}
        for s in range(HALF // 8):
            ks = [k for k in range(NBR)
                  if win[k][0]['a0'] < 8 * s + 8 and win[k][0]['b0'] > 8 * s]
            strip_after[s] = max(ks, key=lambda k: pos[k])

        for k in k_order:
            # k=8 lives until strip 7 at the very end -> own buffer, not the
            # rotating pool (a rotating slot would stall later E allocations).
            et = e_p.tile([128, NXB * EPAD], bf16,
                          tag="E8" if k == 8 else "E")
            e_tiles[k] = et
            for xb in range(NXB):
                w = win[k][xb]
                rows, nx, xlo, a0 = w['rows'], w['nx'], w['xlo'], w['a0']
                ntot = rows * nx
                gp = gps.tile([128, 1024], f32, tag="G")
                for ch in (0, 1):
                    lhsT = r_t[k][ch][:, 128 * xb:128 * xb + 128]
                    for pi, (off, pr) in enumerate(w['pieces']):
                        rhs = t3[ch][:, a0 + off:a0 + off + pr, xlo:xlo + nx]
                        o = gp[:, pi * 512:pi * 512 + pr * nx]
                        nc.tensor.matmul(o, lhsT, rhs, start=(ch == 0),
                                         stop=(ch == 1))
                eo = et[:, xb * EPAD:xb * EPAD + ntot]
                if len(w['pieces']) == 1:
                    ei = gp[:, 0:ntot]
                else:
                    h = w['pieces'][0][1] * nx
                    ei = gp[:].rearrange("p (t c) -> p t c", t=2)[:, :, 0:h]
                    eo = eo.rearrange("p (t c) -> p t c", t=2)
                nc.scalar.activation(eo, ei, Exp, bias=bias_t[:], scale=1.0)
                m_a = a0 - w['ky0'] + 12
                xr = xlo - (16 * xb - 6)
                e3 = et[:, xb * EPAD:xb * EPAD + ntot].rearrange(
                    "p (r x) -> p r x", r=rows)
                nc.vector.tensor_mul(
                    e3, e3, msk3[:, m_a:m_a + rows, xr:xr + nx])
            for s in range(HALF // 8):
                if strip_after[s] == k:
                    do_strip(s)
    nc.compile()
    return nc


def _prep_host(quantized_r):
    q = quantized_r[:, 0]
    a = q.reshape(B, H1, 4, 512)[:, :, 1:3, :].sum(2)
    s = a.reshape(B, H1, W1, 4)[:, :, :, 1:3].sum(3)
    # CPU-jax reference semantics: f32->i32 convert truncates (values >= 0)
    return s // 4


def _mask_host():
    ky = (np.arange(128) // 16)[:, None, None]
    kx = (np.arange(128) % 16)[:, None, None]
    mi = np.arange(32)[None, :, None]
    rx = np.arange(28)[None, None, :]
    m = ((np.abs(mi - 12 - ky) <= 6) & (np.abs(rx - 6 - kx) <= 6))
    return m.astype(np.float32).reshape(128, 32 * 28).astype(ml_dtypes.bfloat16)


def _oht_host(labels_half):
    o = np.zeros((128, NBR * NXB, OW), np.float32)
    for k in range(NBR):
        for xb in range(NXB):
            lab = labels_half[8 * k:8 * k + 8, 16 * xb:16 * xb + 16].reshape(128)
            o[np.arange(128), k * NXB + xb, lab] = 1.0
            o[:, k * NXB + xb, C] = 1.0  # denominator ones column
    return o.reshape(128, NBR * NXB * OW).astype(ml_dtypes.bfloat16)


def kernel(feats_r, feats_t, quantized_r):
    global _COMPILED, _LAST_RES
    import os
    from concourse.bass_utils import run_bass_kernel_spmd

    feats_r = np.asarray(feats_r, np.float32)
    feats_t = np.asarray(feats_t, np.float32)
    quantized_r = np.asarray(quantized_r, np.int32)

    if _COMPILED is None:
        _COMPILED = _build()

    labels = _prep_host(quantized_r)
    msk = _mask_host()
    fr16 = feats_r.astype(np.float16)
    ft16 = feats_t.astype(np.float16)
    in_maps = []
    for core in range(8):
        half, b = core // 4, core % 4
        if half == 0:
            t = ft16[b, :, 0:HALF, :]
            r = fr16[b, :, 0:72, :]
            lab = labels[b, 0:72, :]
        else:  # y-mirrored bottom half
            t = ft16[b, :, ::-1, :][:, 0:HALF, :]
            r = fr16[b, :, ::-1, :][:, 0:72, :]
            lab = labels[b, ::-1, :][0:72, :]
        r_bm = np.ascontiguousarray(r).reshape(CF, NBR, 8, NXB, 16) \
            .transpose(0, 1, 3, 2, 4).reshape(CF, 72 * W1)
        in_maps.append(dict(
            t=np.ascontiguousarray(t.reshape(CF, HALF * W1)),
            r=np.ascontiguousarray(r_bm),
            oht=np.ascontiguousarray(_oht_host(lab)),
            msk=msk,
        ))
    trace = bool(os.environ.get("KPROF"))
    res = run_bass_kernel_spmd(_COMPILED, in_maps, core_ids=list(range(8)),
                               trace=trace)
    _LAST_RES = res
    out = np.empty((B, C, H1, W1), np.float32)
    for core in range(8):
        half, b = core // 4, core % 4
        o = res.results[core]["out"]
        v = o[0:C] / o[C:C + 1]
        if half == 0:
            out[b, :, 0:HALF, :] = v
        else:
            out[b, :, HALF:, :] = v[:, ::-1, :]
    return out
